# revision 21
# baseline (speedup 1.0000x reference)
"""ColumnParallelFusedMoeLinear grouped-GEMM kernel for 8 Trainium2 NeuronCores.

Strategy (expert/token parallel):
  Tokens are sorted by expert; m_sizes gives each expert's contiguous row
  range of x.  The host assigns each of the 8 cores one contiguous
  single-expert chunk of up to 1024 tokens (large experts get split
  across cores when slots are free).  Each core computes its chunk's
  y_chunk = x_chunk @ weight[e].T and the host scatters rows back.  The
  few rows that don't fit the 8x1024 device capacity (~1% for balanced
  m_sizes) are computed on host in fp32 BLAS.

  Design notes (vs the 87.2us m_pad=1068 baseline):
  * m_pad == 1024 EXACTLY: every PSUM block is a full 512-col bank and
    every matmul is a full-width 512-column pass -- no 48-col runt MM
    per (nt, kc) group (the runt slot cost ~44ns x 128 groups ~ 5.6us).
  * t0/t1/t2 INTERLEAVED warm-up phase: the x fill (2 MB on the sync
    ring) paces the start; processing the first THREE output tiles
    kc-by-kc as chunks arrive gives the PE ~1.3us of work per 0.76us
    arrival, so it never idles long enough for the HAM clock-gate to
    re-throttle mid-fill.
  * The first three weight groups load as TWO kc-half tiles each,
    interleaved with the x chunks (w0a x0 w1a x1 w2a x2 x3 w0b x4 w1b
    x5 w2b ...): the fill head stays small so the early x chunks land
    ~0.8us sooner, and each half arrives just ahead of its first
    matmul.  Separate whole tiles (not partial-tile writes, which
    serialize pathologically; not column sub-ranges, which stride).
  * WEIGHT-stationary matmuls: lhsT = wT tile [128k x 128 d_out], moving
    rhs = xT [128k x m tokens].
  * Weights HOST-PACKED into the exact SBUF layout so weight DMAs are
    contiguous per-partition runs.
  * Output written as yT [d_out, m_pad] bf16 (psum holds yT tiles);
    host transposes back.  Stores stream per nt-pair on the scalar
    HWDGE ring during compute; the last nt stores per-256-col pieces so
    the final store (and its HBM receipt) is small.
  * PE warm-up: dummy matmuls on a memset scratch tile run during the
    initial DMA fill so the HAM clock-gate is at 8/8 when real work
    lands.
  * An fp8-e4m3 DoubleRow path (kc6,7 fused into one double-pumped
    slot) is kept behind MOE_FP8=1.  It is OFF by default: on this part
    the chip's power manager responds to the densified MAC stream by
    dropping the PE clock 2.4 -> 2.0 GHz (P0), which more than cancels
    the arithmetic win.
"""

import math
import os

import numpy as np

_N_CORES = 8
_P = 128
_MBLK = 512   # PSUM bank width in fp32
_MCHUNK = 1024  # device tokens per core chunk (2 full PSUM banks)

_FP8 = os.environ.get("MOE_FP8", "0") == "1"
_KC_F8 = 2 if _FP8 else 0   # contraction chunks done in fp8 DoubleRow
_F8_SHIFT = 4.0             # w_f8 *= shift, x_f8 /= shift (exact pow2)

_program_cache = {}


def _build_program(m_pad, d_in, d_out, warm_mms=8):
    import concourse.mybir as mybir
    import concourse.tile as tile
    from concourse import bacc

    kc_n = d_in // _P            # total contraction chunks of 128
    kc_bf = kc_n - _KC_F8        # bf16 chunks (0..kc_bf-1)
    nt_n = d_out // _P           # output-feature tiles of 128
    blocks = [(s, min(s + _MBLK, m_pad)) for s in range(0, m_pad, _MBLK)]
    nblk = len(blocks)

    nc = bacc.Bacc("TRN2", target_bir_lowering=False, debug=False)
    xT = nc.dram_tensor("xT", [kc_bf * _P, m_pad], mybir.dt.bfloat16,
                        kind="ExternalInput")
    # wP: host-packed bf16 weights. wP[p, (nt*kc_bf + kc)*128 + j] =
    #     weight[e].T[kc*128 + p, nt*128 + j]
    wP = nc.dram_tensor("wP", [_P, nt_n * kc_bf * _P], mybir.dt.bfloat16,
                        kind="ExternalInput")
    if _KC_F8:
        xF8 = nc.dram_tensor("xF8", [_P, _KC_F8 * m_pad], mybir.dt.float8e4,
                             kind="ExternalInput")
        wF8 = nc.dram_tensor("wF8", [_P, _KC_F8 * nt_n * _P],
                             mybir.dt.float8e4, kind="ExternalInput")
    yT = nc.dram_tensor("yT", [d_out, m_pad], mybir.dt.bfloat16,
                        kind="ExternalOutput")

    xT3 = xT.rearrange("(kc p) m -> kc p m", p=_P)
    yT3 = yT.rearrange("(nt p) m -> nt p m", p=_P)

    # weight DMA groups (by nt): singles up front so the first matmuls
    # aren't gated on a big transfer, pairs after.
    wgroups = []
    nt = 0
    while nt < nt_n:
        g = 1 if len(wgroups) < 4 else 2
        g = min(g, nt_n - nt)
        wgroups.append((nt, nt + g))
        nt += g
    gi_of_nt = {}
    for gi, (n0, n1) in enumerate(wgroups):
        for t in range(n0, n1):
            gi_of_nt[t] = gi

    with tile.TileContext(nc) as tc:
        with (
            tc.tile_pool(name="xw", bufs=1) as xwpool,
            tc.tile_pool(name="out", bufs=4) as outpool,
            tc.tile_pool(name="psum", bufs=7, space="PSUM") as psumpool,
            tc.tile_pool(name="wps", bufs=1, space="PSUM") as wpspool,
        ):
            # ---- PE warm-up on a zeroed scratch tile (no data deps) ----
            if warm_mms:
                # vector memset (not gpsimd): DVE clears the entry barrier
                # first and finishes the fill ~0.5us sooner, so the HAM
                # warm-up matmuls start earlier.
                scratch = xwpool.tile([_P, 640], mybir.dt.bfloat16, tag="scratch")
                nc.vector.memset(scratch[:], 0.0)
                wps = wpspool.tile([_P, _MBLK], mybir.dt.float32, tag="wps")
                for _ in range(warm_mms):
                    nc.tensor.matmul(wps[:], scratch[:, 0:_P], scratch[:, _P:640],
                                     start=True, stop=True)

            # ---- input DMAs (sync/SP HWDGE ring), in PE need-order ----
            xsb = xwpool.tile([_P, kc_bf * m_pad], mybir.dt.bfloat16, tag="x")
            wsb = {}

            kc_s = kc_bf // 2  # split point for the early half-loaded groups
            wsb_half = {}

            def load_w(gi):
                n0, n1 = wgroups[gi]
                t = xwpool.tile([_P, (n1 - n0) * kc_bf * _P], mybir.dt.bfloat16,
                                tag=f"w{gi}")
                nc.sync.dma_start(t[:], wP[:, n0 * kc_bf * _P:n1 * kc_bf * _P])
                wsb[gi] = t

            def load_w_half(gi, half):
                # the first interleaved groups load in two kc-halves (each a
                # separate whole tile -> clean deps, contiguous DMA) so the
                # head of the fill is small and the early x chunks land
                # sooner.
                n0 = wgroups[gi][0]
                lo, hi = (0, kc_s) if half == 0 else (kc_s, kc_bf)
                t = xwpool.tile([_P, (hi - lo) * _P], mybir.dt.bfloat16,
                                tag=f"w{gi}h{half}")
                base = n0 * kc_bf * _P
                nc.sync.dma_start(t[:], wP[:, base + lo * _P:base + hi * _P])
                wsb_half[(gi, half)] = t

            def load_x(kc):
                nc.sync.dma_start(xsb[:, kc * m_pad:(kc + 1) * m_pad], xT3[kc])

            n_split = min(3, len(wgroups))
            load_w_half(0, 0)
            load_x(0)
            if n_split > 1:
                load_w_half(1, 0)
            load_x(1)
            if n_split > 2:
                load_w_half(2, 0)
            load_x(2)
            if _KC_F8:
                xf8 = xwpool.tile([_P, _KC_F8, m_pad], mybir.dt.float8e4,
                                  tag="xf8")
                wf8 = xwpool.tile([_P, _KC_F8, nt_n * _P], mybir.dt.float8e4,
                                  tag="wf8")
                xF83 = xF8.rearrange("p (k m) -> p k m", k=_KC_F8)
                wF83 = wF8.rearrange("p (k c) -> p k c", k=_KC_F8)
                nc.sync.dma_start(xf8[:, :, :], xF83[:, :, :])
            load_x(3)
            load_w_half(0, 1)
            load_x(4)
            if n_split > 1:
                load_w_half(1, 1)
            if _KC_F8:
                nc.sync.dma_start(wf8[:, :, 0:4 * _P], wF83[:, :, 0:4 * _P])
            load_x(5)
            if n_split > 2:
                load_w_half(2, 1)
            for kc in range(6, kc_bf):
                load_x(kc)
            if _KC_F8:
                nc.sync.dma_start(wf8[:, :, 4 * _P:], wF83[:, :, 4 * _P:])
            for gi in range(n_split, len(wgroups)):
                load_w(gi)

            # ---- compute + cast + store ----
            def emit_bf16_kc(t, kc, ps):
                gi = gi_of_nt[t]
                if gi < n_split:
                    half = 0 if kc < kc_s else 1
                    off = (kc - (0 if half == 0 else kc_s)) * _P
                    lhsT = wsb_half[(gi, half)][:, off:off + _P]
                else:
                    n0 = wgroups[gi][0]
                    off = ((t - n0) * kc_bf + kc) * _P
                    lhsT = wsb[gi][:, off:off + _P]
                last_kc = (kc == kc_bf - 1) and not _KC_F8
                for bi, (s, e) in enumerate(blocks):
                    nc.tensor.matmul(
                        ps[bi][:, :e - s],
                        lhsT,
                        xsb[:, kc * m_pad + s:kc * m_pad + e],
                        start=(kc == 0),
                        stop=last_kc,
                    )

            def emit_dr(t, ps):
                lhsT8 = wf8[:, :, t * _P:(t + 1) * _P]
                for bi, (s, e) in enumerate(blocks):
                    nc.tensor.matmul(
                        ps[bi][:, :e - s],
                        lhsT8,
                        xf8[:, :, s:e],
                        start=False,
                        stop=True,
                        perf_mode=mybir.MatmulPerfMode.DoubleRow,
                    )

            pend_pair = {}

            def emit_output(t, ps):
                # stores per nt-pair; the final nt per-256-col piece so the
                # last store (and its HBM receipt) is small.
                last = (t == nt_n - 1)
                paired = (t // 2) * 2 + 1 <= nt_n - 2
                if not last:
                    if not paired:
                        o = outpool.tile([_P, m_pad], mybir.dt.bfloat16,
                                         tag="o", name=f"o_{t}")
                        o_half = 0
                    elif t % 2 == 0:
                        o = outpool.tile([_P, 2 * m_pad], mybir.dt.bfloat16,
                                         tag="o", name=f"o_{t}")
                        pend_pair[t] = o
                        o_half = 0
                    else:
                        o = pend_pair.pop(t - 1)
                        o_half = 1
                    dst = o[:, o_half * m_pad:(o_half + 1) * m_pad]
                    for bi, (s, e) in enumerate(blocks):
                        nc.vector.tensor_copy(dst[:, s:e], ps[bi][:, :e - s])
                    if paired and t % 2 == 1:
                        nc.scalar.dma_start(
                            yT3[t - 1:t + 1].rearrange("t p m -> p t m"),
                            o[:].rearrange("p (t m) -> p t m", t=2),
                        )
                    elif not paired:
                        nc.scalar.dma_start(yT3[t], o[:])
                else:
                    # last nt: per-block cast; the final block's store is
                    # split across BOTH HWDGE rings (sync is idle by now)
                    # so the two closing receipts overlap.
                    ol = outpool.tile([_P, m_pad], mybir.dt.bfloat16,
                                      tag="ol", name="o_last")
                    for bi, (s, e) in enumerate(blocks):
                        nc.vector.tensor_copy(
                            ol[:, s:e], ps[bi][:, :e - s])
                        if bi < nblk - 1:
                            nc.sync.dma_start(yT3[t][:, s:e], ol[:, s:e])
                        else:
                            mid = (s + e) // 2
                            nc.scalar.dma_start(yT3[t][:, s:mid], ol[:, s:mid])
                            nc.sync.dma_start(yT3[t][:, mid:e], ol[:, mid:e])

            def make_ps(t):
                return [psumpool.tile([_P, _MBLK], mybir.dt.float32,
                                      tag="ps", name=f"ps_{t}_{bi}")
                        for bi in range(nblk)]

            # phase 1: first few nt interleaved kc-by-kc (PE stays busy
            # while the x chunks stream in)
            n_inter = 3 if nblk <= 2 else 2
            n_inter = min(n_inter, nt_n)
            ps_i = {t: make_ps(t) for t in range(n_inter)}
            for kc in range(kc_bf):
                for t in range(n_inter):
                    emit_bf16_kc(t, kc, ps_i[t])
            for t in range(n_inter):
                if _KC_F8:
                    emit_dr(t, ps_i[t])
                emit_output(t, ps_i[t])

            # phase 2: remaining nt, one at a time
            for t in range(n_inter, nt_n):
                ps = make_ps(t)
                for kc in range(kc_bf):
                    emit_bf16_kc(t, kc, ps)
                if _KC_F8:
                    emit_dr(t, ps)
                emit_output(t, ps)
    nc.compile()
    return nc


def _plan_chunks(m_sizes, T):
    """Assign each core one contiguous single-expert chunk of <= _MCHUNK
    rows, maximizing device coverage; whatever doesn't fit the 8 slots
    goes to the host-GEMM leftover list.

    Returns (chunks, host_segs): chunks is a list of _N_CORES
    (expert, row0, row1) tuples (possibly empty (0,0,0)); host_segs is a
    list of (expert, row0, row1) computed on host.
    """
    off = np.cumsum(np.asarray(m_sizes, dtype=np.int64))
    starts = np.clip(np.concatenate([[0], off[:-1]]), 0, T)
    ends = np.clip(off, 0, T)
    segs = [(e, int(starts[e]), int(ends[e]))
            for e in range(len(m_sizes)) if ends[e] > starts[e]]
    if not segs:
        return None, []
    # slots per expert: start with 1 each (if it fits), then give spare
    # slots to the experts with the largest remaining overflow.
    nseg = len(segs)
    if nseg > _N_CORES:
        # more experts than cores: biggest experts get the slots, the
        # rest go to host entirely.
        segs_sorted = sorted(segs, key=lambda s: s[2] - s[1], reverse=True)
        host = segs_sorted[_N_CORES:]
        segs = [s for s in segs if s in segs_sorted[:_N_CORES]]
    else:
        host = []
    slots = {i: 1 for i in range(len(segs))}
    spare = _N_CORES - len(segs)
    for _ in range(spare):
        ovf = [(segs[i][2] - segs[i][1]) - slots[i] * _MCHUNK
               for i in range(len(segs))]
        j = int(np.argmax(ovf))
        if ovf[j] <= 0:
            break
        slots[j] += 1
    chunks = []
    host_segs = list(host)
    for i, (e, s0, s1) in enumerate(segs):
        L = s1 - s0
        dev = min(L, slots[i] * _MCHUNK)
        k = slots[i]
        bounds = [s0 + (dev * q) // k for q in range(k + 1)]
        for q in range(k):
            if bounds[q + 1] > bounds[q]:
                chunks.append((e, bounds[q], bounds[q + 1]))
        if dev < L:
            host_segs.append((e, s0 + dev, s1))
    while len(chunks) < _N_CORES:
        chunks.append((0, 0, 0))
    return chunks, host_segs


def _pack_weight_bf(wT_e, kc_bf, nt_n, bf16):
    """wT_e [d_in, d_out] -> [128, nt*kc_bf*128] bf16 in the SBUF layout."""
    w4 = wT_e[:kc_bf * _P].reshape(kc_bf, _P, nt_n, _P)   # [kc, p, nt, j]
    return np.ascontiguousarray(
        w4.transpose(1, 2, 0, 3).reshape(_P, nt_n * kc_bf * _P).astype(bf16))


def _pack_weight_f8(wT_e, kc_bf, nt_n, f8):
    """wT_e last _KC_F8 kc chunks -> [128, _KC_F8*nt_n*128] fp8 (ksub-major)."""
    w3 = (wT_e[kc_bf * _P:] * _F8_SHIFT).reshape(_KC_F8, _P, nt_n * _P)
    return np.ascontiguousarray(
        w3.transpose(1, 0, 2).reshape(_P, _KC_F8 * nt_n * _P).astype(f8))


def kernel(x, weight, m_sizes):
    import ml_dtypes
    from concourse.bass_utils import run_bass_kernel_spmd

    bf16 = ml_dtypes.bfloat16
    f8 = ml_dtypes.float8_e4m3
    x = np.ascontiguousarray(np.asarray(x), dtype=np.float32)
    weight = np.ascontiguousarray(np.asarray(weight), dtype=np.float32)
    m_arr = np.asarray(m_sizes)

    T, d_in = x.shape
    E, d_out, _ = weight.shape
    kc_n = d_in // _P
    kc_bf = kc_n - _KC_F8
    nt_n = d_out // _P

    y = np.zeros((T, d_out), dtype=np.float32)
    chunks, host_segs = _plan_chunks(m_arr, T)
    if chunks is None:
        return y

    m_pad = _MCHUNK

    warm_mms = int(os.environ.get("MOE_WARM_MMS", "8"))
    key = (m_pad, d_in, d_out, warm_mms, _KC_F8)
    if key not in _program_cache:
        _program_cache[key] = _build_program(m_pad, d_in, d_out, warm_mms)
    nc = _program_cache[key]

    wP_cache = {}
    in_maps = []
    for e, r0, r1 in chunks:
        xTc = np.zeros((d_in, m_pad), dtype=np.float32)
        if r1 > r0:
            xTc[:, : r1 - r0] = x[r0:r1].T
        if e not in wP_cache:
            wT_e = np.ascontiguousarray(weight[e].T)
            wP_cache[e] = (
                _pack_weight_bf(wT_e, kc_bf, nt_n, bf16),
                _pack_weight_f8(wT_e, kc_bf, nt_n, f8) if _KC_F8 else None,
            )
        wPb, wPf = wP_cache[e]
        im = {"xT": xTc[:kc_bf * _P].astype(bf16), "wP": wPb}
        if _KC_F8:
            im["xF8"] = np.ascontiguousarray(
                (xTc[kc_bf * _P:] * (1.0 / _F8_SHIFT))
                .reshape(_KC_F8, _P, m_pad).transpose(1, 0, 2)
                .reshape(_P, _KC_F8 * m_pad)).astype(f8)
            im["wF8"] = wPf
        in_maps.append(im)

    res = run_bass_kernel_spmd(nc, in_maps, core_ids=list(range(_N_CORES)))

    for (e, r0, r1), out in zip(chunks, res.results):
        if r1 > r0:
            y[r0:r1] = out["yT"][:, : r1 - r0].T.astype(np.float32)

    # leftover rows (device capacity is 8 x 1024): host fp32 BLAS
    for e, r0, r1 in host_segs:
        y[r0:r1] = x[r0:r1] @ weight[e].T
    return y


# revision 22
# speedup vs baseline: 1.0037x; 1.0037x over previous
"""ColumnParallelFusedMoeLinear grouped-GEMM kernel for 8 Trainium2 NeuronCores.

Strategy (expert/token parallel):
  Tokens are sorted by expert; m_sizes gives each expert's contiguous row
  range of x.  The host assigns each of the 8 cores one contiguous
  single-expert chunk of up to 1024 tokens (large experts get split
  across cores when slots are free).  Each core computes its chunk's
  y_chunk = x_chunk @ weight[e].T and the host scatters rows back.  The
  few rows that don't fit the 8x1024 device capacity (~1% for balanced
  m_sizes) are computed on host in fp32 BLAS.

  Design notes (vs the 87.2us m_pad=1068 baseline):
  * m_pad == 1024 EXACTLY: every PSUM block is a full 512-col bank and
    every matmul is a full-width 512-column pass -- no 48-col runt MM
    per (nt, kc) group (the runt slot cost ~44ns x 128 groups ~ 5.6us).
  * t0/t1/t2 INTERLEAVED warm-up phase: the x fill (2 MB on the sync
    ring) paces the start; processing the first THREE output tiles
    kc-by-kc as chunks arrive gives the PE ~1.3us of work per 0.76us
    arrival, so it never idles long enough for the HAM clock-gate to
    re-throttle mid-fill.
  * The first three weight groups load as TWO kc-half tiles each,
    interleaved with the x chunks (w0a x0 w1a x1 w2a x2 x3 w0b x4 w1b
    x5 w2b ...): the fill head stays small so the early x chunks land
    ~0.8us sooner, and each half arrives just ahead of its first
    matmul.  Separate whole tiles (not partial-tile writes, which
    serialize pathologically; not column sub-ranges, which stride).
  * WEIGHT-stationary matmuls: lhsT = wT tile [128k x 128 d_out], moving
    rhs = xT [128k x m tokens].
  * Weights HOST-PACKED into the exact SBUF layout so weight DMAs are
    contiguous per-partition runs.
  * Output written as yT [d_out, m_pad] bf16 (psum holds yT tiles);
    host transposes back.  Stores stream per nt-pair on the scalar
    HWDGE ring during compute; the last nt stores per-256-col pieces so
    the final store (and its HBM receipt) is small.
  * PE warm-up: dummy matmuls on a memset scratch tile run during the
    initial DMA fill so the HAM clock-gate is at 8/8 when real work
    lands.
  * An fp8-e4m3 DoubleRow path (kc6,7 fused into one double-pumped
    slot) is kept behind MOE_FP8=1.  It is OFF by default: on this part
    the chip's power manager responds to the densified MAC stream by
    dropping the PE clock 2.4 -> 2.0 GHz (P0), which more than cancels
    the arithmetic win.
"""

import math
import os

import numpy as np

_N_CORES = 8
_P = 128
_MBLK = 512   # PSUM bank width in fp32
_MCHUNK = 1024  # device tokens per core chunk (2 full PSUM banks)

_FP8 = os.environ.get("MOE_FP8", "0") == "1"
_KC_F8 = 2 if _FP8 else 0   # contraction chunks done in fp8 DoubleRow
_F8_SHIFT = 4.0             # w_f8 *= shift, x_f8 /= shift (exact pow2)

_program_cache = {}


def _build_program(m_pad, d_in, d_out, warm_mms=8):
    import concourse.mybir as mybir
    import concourse.tile as tile
    from concourse import bacc

    kc_n = d_in // _P            # total contraction chunks of 128
    kc_bf = kc_n - _KC_F8        # bf16 chunks (0..kc_bf-1)
    nt_n = d_out // _P           # output-feature tiles of 128
    blocks = [(s, min(s + _MBLK, m_pad)) for s in range(0, m_pad, _MBLK)]
    nblk = len(blocks)

    nc = bacc.Bacc("TRN2", target_bir_lowering=False, debug=False)
    xT = nc.dram_tensor("xT", [kc_bf * _P, m_pad], mybir.dt.bfloat16,
                        kind="ExternalInput")
    # wP: host-packed bf16 weights. wP[p, (nt*kc_bf + kc)*128 + j] =
    #     weight[e].T[kc*128 + p, nt*128 + j]
    wP = nc.dram_tensor("wP", [_P, nt_n * kc_bf * _P], mybir.dt.bfloat16,
                        kind="ExternalInput")
    if _KC_F8:
        xF8 = nc.dram_tensor("xF8", [_P, _KC_F8 * m_pad], mybir.dt.float8e4,
                             kind="ExternalInput")
        wF8 = nc.dram_tensor("wF8", [_P, _KC_F8 * nt_n * _P],
                             mybir.dt.float8e4, kind="ExternalInput")
    yT = nc.dram_tensor("yT", [d_out, m_pad], mybir.dt.bfloat16,
                        kind="ExternalOutput")

    xT3 = xT.rearrange("(kc p) m -> kc p m", p=_P)
    yT3 = yT.rearrange("(nt p) m -> nt p m", p=_P)

    # weight DMA groups (by nt): singles up front so the first matmuls
    # aren't gated on a big transfer, pairs after.
    wgroups = []
    nt = 0
    while nt < nt_n:
        g = 1 if len(wgroups) < 4 else 2
        g = min(g, nt_n - nt)
        wgroups.append((nt, nt + g))
        nt += g
    gi_of_nt = {}
    for gi, (n0, n1) in enumerate(wgroups):
        for t in range(n0, n1):
            gi_of_nt[t] = gi

    with tile.TileContext(nc) as tc:
        with (
            tc.tile_pool(name="xw", bufs=1) as xwpool,
            tc.tile_pool(name="out", bufs=4) as outpool,
            tc.tile_pool(name="psum", bufs=7, space="PSUM") as psumpool,
            tc.tile_pool(name="wps", bufs=1, space="PSUM") as wpspool,
        ):
            # ---- PE warm-up on a zeroed scratch tile (no data deps) ----
            if warm_mms:
                # vector memset (not gpsimd): DVE clears the entry barrier
                # first and finishes the fill ~0.5us sooner, so the HAM
                # warm-up matmuls start earlier.
                scratch = xwpool.tile([_P, 640], mybir.dt.bfloat16, tag="scratch")
                nc.vector.memset(scratch[:], 0.0)
                wps = wpspool.tile([_P, _MBLK], mybir.dt.float32, tag="wps")
                for _ in range(warm_mms):
                    nc.tensor.matmul(wps[:], scratch[:, 0:_P], scratch[:, _P:640],
                                     start=True, stop=True)

            # ---- input DMAs (sync/SP HWDGE ring), in PE need-order ----
            xsb = xwpool.tile([_P, kc_bf * m_pad], mybir.dt.bfloat16, tag="x")
            wsb = {}

            kc_s = kc_bf // 2  # split point for the early half-loaded groups
            wsb_half = {}

            def load_w(gi):
                n0, n1 = wgroups[gi]
                t = xwpool.tile([_P, (n1 - n0) * kc_bf * _P], mybir.dt.bfloat16,
                                tag=f"w{gi}")
                nc.sync.dma_start(t[:], wP[:, n0 * kc_bf * _P:n1 * kc_bf * _P])
                wsb[gi] = t

            def load_w_half(gi, half):
                # the first interleaved groups load in two kc-halves (each a
                # separate whole tile -> clean deps, contiguous DMA) so the
                # head of the fill is small and the early x chunks land
                # sooner.
                n0 = wgroups[gi][0]
                lo, hi = (0, kc_s) if half == 0 else (kc_s, kc_bf)
                t = xwpool.tile([_P, (hi - lo) * _P], mybir.dt.bfloat16,
                                tag=f"w{gi}h{half}")
                base = n0 * kc_bf * _P
                nc.sync.dma_start(t[:], wP[:, base + lo * _P:base + hi * _P])
                wsb_half[(gi, half)] = t

            def load_x(kc):
                nc.sync.dma_start(xsb[:, kc * m_pad:(kc + 1) * m_pad], xT3[kc])

            n_split = min(3, len(wgroups))
            load_w_half(0, 0)
            load_x(0)
            if n_split > 1:
                load_w_half(1, 0)
            load_x(1)
            if n_split > 2:
                load_w_half(2, 0)
            load_x(2)
            if _KC_F8:
                xf8 = xwpool.tile([_P, _KC_F8, m_pad], mybir.dt.float8e4,
                                  tag="xf8")
                wf8 = xwpool.tile([_P, _KC_F8, nt_n * _P], mybir.dt.float8e4,
                                  tag="wf8")
                xF83 = xF8.rearrange("p (k m) -> p k m", k=_KC_F8)
                wF83 = wF8.rearrange("p (k c) -> p k c", k=_KC_F8)
                nc.sync.dma_start(xf8[:, :, :], xF83[:, :, :])
            load_x(3)
            load_w_half(0, 1)
            load_x(4)
            if n_split > 1:
                load_w_half(1, 1)
            if _KC_F8:
                nc.sync.dma_start(wf8[:, :, 0:4 * _P], wF83[:, :, 0:4 * _P])
            load_x(5)
            if n_split > 2:
                load_w_half(2, 1)
            for kc in range(6, kc_bf):
                load_x(kc)
            if _KC_F8:
                nc.sync.dma_start(wf8[:, :, 4 * _P:], wF83[:, :, 4 * _P:])
            # w3 (first phase-2 tile) also loads as halves: its kc0-3 half
            # arrives ~0.3us sooner, widening the tightest phase-2 handoff.
            n_half2 = min(4, len(wgroups))
            for gi in range(n_split, n_half2):
                load_w_half(gi, 0)
                load_w_half(gi, 1)
            for gi in range(n_half2, len(wgroups)):
                load_w(gi)

            # ---- compute + cast + store ----
            def emit_bf16_kc(t, kc, ps):
                gi = gi_of_nt[t]
                if (gi, 0) in wsb_half:
                    half = 0 if kc < kc_s else 1
                    off = (kc - (0 if half == 0 else kc_s)) * _P
                    lhsT = wsb_half[(gi, half)][:, off:off + _P]
                else:
                    n0 = wgroups[gi][0]
                    off = ((t - n0) * kc_bf + kc) * _P
                    lhsT = wsb[gi][:, off:off + _P]
                last_kc = (kc == kc_bf - 1) and not _KC_F8
                for bi, (s, e) in enumerate(blocks):
                    nc.tensor.matmul(
                        ps[bi][:, :e - s],
                        lhsT,
                        xsb[:, kc * m_pad + s:kc * m_pad + e],
                        start=(kc == 0),
                        stop=last_kc,
                    )

            def emit_dr(t, ps):
                lhsT8 = wf8[:, :, t * _P:(t + 1) * _P]
                for bi, (s, e) in enumerate(blocks):
                    nc.tensor.matmul(
                        ps[bi][:, :e - s],
                        lhsT8,
                        xf8[:, :, s:e],
                        start=False,
                        stop=True,
                        perf_mode=mybir.MatmulPerfMode.DoubleRow,
                    )

            pend_pair = {}

            def emit_output(t, ps):
                # stores per nt-pair; the final nt per-256-col piece so the
                # last store (and its HBM receipt) is small.
                last = (t == nt_n - 1)
                paired = (t // 2) * 2 + 1 <= nt_n - 2
                if not last:
                    if not paired:
                        o = outpool.tile([_P, m_pad], mybir.dt.bfloat16,
                                         tag="o", name=f"o_{t}")
                        o_half = 0
                    elif t % 2 == 0:
                        o = outpool.tile([_P, 2 * m_pad], mybir.dt.bfloat16,
                                         tag="o", name=f"o_{t}")
                        pend_pair[t] = o
                        o_half = 0
                    else:
                        o = pend_pair.pop(t - 1)
                        o_half = 1
                    dst = o[:, o_half * m_pad:(o_half + 1) * m_pad]
                    for bi, (s, e) in enumerate(blocks):
                        nc.vector.tensor_copy(dst[:, s:e], ps[bi][:, :e - s])
                    if paired and t % 2 == 1:
                        nc.scalar.dma_start(
                            yT3[t - 1:t + 1].rearrange("t p m -> p t m"),
                            o[:].rearrange("p (t m) -> p t m", t=2),
                        )
                    elif not paired:
                        nc.scalar.dma_start(yT3[t], o[:])
                else:
                    # last nt: per-block cast; the final block's store is
                    # split across BOTH HWDGE rings (sync is idle by now)
                    # so the two closing receipts overlap.
                    ol = outpool.tile([_P, m_pad], mybir.dt.bfloat16,
                                      tag="ol", name="o_last")
                    for bi, (s, e) in enumerate(blocks):
                        nc.vector.tensor_copy(
                            ol[:, s:e], ps[bi][:, :e - s])
                        if bi < nblk - 1:
                            nc.sync.dma_start(yT3[t][:, s:e], ol[:, s:e])
                        else:
                            mid = (s + e) // 2
                            nc.scalar.dma_start(yT3[t][:, s:mid], ol[:, s:mid])
                            nc.sync.dma_start(yT3[t][:, mid:e], ol[:, mid:e])

            def make_ps(t):
                return [psumpool.tile([_P, _MBLK], mybir.dt.float32,
                                      tag="ps", name=f"ps_{t}_{bi}")
                        for bi in range(nblk)]

            # phase 1: first few nt interleaved kc-by-kc (PE stays busy
            # while the x chunks stream in)
            n_inter = 3 if nblk <= 2 else 2
            n_inter = min(n_inter, nt_n)
            ps_i = {t: make_ps(t) for t in range(n_inter)}
            for kc in range(kc_bf):
                for t in range(n_inter):
                    emit_bf16_kc(t, kc, ps_i[t])
            for t in range(n_inter):
                if _KC_F8:
                    emit_dr(t, ps_i[t])
                emit_output(t, ps_i[t])

            # phase 2: remaining nt, one at a time
            for t in range(n_inter, nt_n):
                ps = make_ps(t)
                for kc in range(kc_bf):
                    emit_bf16_kc(t, kc, ps)
                if _KC_F8:
                    emit_dr(t, ps)
                emit_output(t, ps)
    nc.compile()
    return nc


def _plan_chunks(m_sizes, T):
    """Assign each core one contiguous single-expert chunk of <= _MCHUNK
    rows, maximizing device coverage; whatever doesn't fit the 8 slots
    goes to the host-GEMM leftover list.

    Returns (chunks, host_segs): chunks is a list of _N_CORES
    (expert, row0, row1) tuples (possibly empty (0,0,0)); host_segs is a
    list of (expert, row0, row1) computed on host.
    """
    off = np.cumsum(np.asarray(m_sizes, dtype=np.int64))
    starts = np.clip(np.concatenate([[0], off[:-1]]), 0, T)
    ends = np.clip(off, 0, T)
    segs = [(e, int(starts[e]), int(ends[e]))
            for e in range(len(m_sizes)) if ends[e] > starts[e]]
    if not segs:
        return None, []
    # slots per expert: start with 1 each (if it fits), then give spare
    # slots to the experts with the largest remaining overflow.
    nseg = len(segs)
    if nseg > _N_CORES:
        # more experts than cores: biggest experts get the slots, the
        # rest go to host entirely.
        segs_sorted = sorted(segs, key=lambda s: s[2] - s[1], reverse=True)
        host = segs_sorted[_N_CORES:]
        segs = [s for s in segs if s in segs_sorted[:_N_CORES]]
    else:
        host = []
    slots = {i: 1 for i in range(len(segs))}
    spare = _N_CORES - len(segs)
    for _ in range(spare):
        ovf = [(segs[i][2] - segs[i][1]) - slots[i] * _MCHUNK
               for i in range(len(segs))]
        j = int(np.argmax(ovf))
        if ovf[j] <= 0:
            break
        slots[j] += 1
    chunks = []
    host_segs = list(host)
    for i, (e, s0, s1) in enumerate(segs):
        L = s1 - s0
        dev = min(L, slots[i] * _MCHUNK)
        k = slots[i]
        bounds = [s0 + (dev * q) // k for q in range(k + 1)]
        for q in range(k):
            if bounds[q + 1] > bounds[q]:
                chunks.append((e, bounds[q], bounds[q + 1]))
        if dev < L:
            host_segs.append((e, s0 + dev, s1))
    while len(chunks) < _N_CORES:
        chunks.append((0, 0, 0))
    return chunks, host_segs


def _pack_weight_bf(wT_e, kc_bf, nt_n, bf16):
    """wT_e [d_in, d_out] -> [128, nt*kc_bf*128] bf16 in the SBUF layout."""
    w4 = wT_e[:kc_bf * _P].reshape(kc_bf, _P, nt_n, _P)   # [kc, p, nt, j]
    return np.ascontiguousarray(
        w4.transpose(1, 2, 0, 3).reshape(_P, nt_n * kc_bf * _P).astype(bf16))


def _pack_weight_f8(wT_e, kc_bf, nt_n, f8):
    """wT_e last _KC_F8 kc chunks -> [128, _KC_F8*nt_n*128] fp8 (ksub-major)."""
    w3 = (wT_e[kc_bf * _P:] * _F8_SHIFT).reshape(_KC_F8, _P, nt_n * _P)
    return np.ascontiguousarray(
        w3.transpose(1, 0, 2).reshape(_P, _KC_F8 * nt_n * _P).astype(f8))


def kernel(x, weight, m_sizes):
    import ml_dtypes
    from concourse.bass_utils import run_bass_kernel_spmd

    bf16 = ml_dtypes.bfloat16
    f8 = ml_dtypes.float8_e4m3
    x = np.ascontiguousarray(np.asarray(x), dtype=np.float32)
    weight = np.ascontiguousarray(np.asarray(weight), dtype=np.float32)
    m_arr = np.asarray(m_sizes)

    T, d_in = x.shape
    E, d_out, _ = weight.shape
    kc_n = d_in // _P
    kc_bf = kc_n - _KC_F8
    nt_n = d_out // _P

    y = np.zeros((T, d_out), dtype=np.float32)
    chunks, host_segs = _plan_chunks(m_arr, T)
    if chunks is None:
        return y

    m_pad = _MCHUNK

    warm_mms = int(os.environ.get("MOE_WARM_MMS", "8"))
    key = (m_pad, d_in, d_out, warm_mms, _KC_F8)
    if key not in _program_cache:
        _program_cache[key] = _build_program(m_pad, d_in, d_out, warm_mms)
    nc = _program_cache[key]

    wP_cache = {}
    in_maps = []
    for e, r0, r1 in chunks:
        xTc = np.zeros((d_in, m_pad), dtype=np.float32)
        if r1 > r0:
            xTc[:, : r1 - r0] = x[r0:r1].T
        if e not in wP_cache:
            wT_e = np.ascontiguousarray(weight[e].T)
            wP_cache[e] = (
                _pack_weight_bf(wT_e, kc_bf, nt_n, bf16),
                _pack_weight_f8(wT_e, kc_bf, nt_n, f8) if _KC_F8 else None,
            )
        wPb, wPf = wP_cache[e]
        im = {"xT": xTc[:kc_bf * _P].astype(bf16), "wP": wPb}
        if _KC_F8:
            im["xF8"] = np.ascontiguousarray(
                (xTc[kc_bf * _P:] * (1.0 / _F8_SHIFT))
                .reshape(_KC_F8, _P, m_pad).transpose(1, 0, 2)
                .reshape(_P, _KC_F8 * m_pad)).astype(f8)
            im["wF8"] = wPf
        in_maps.append(im)

    res = run_bass_kernel_spmd(nc, in_maps, core_ids=list(range(_N_CORES)))

    for (e, r0, r1), out in zip(chunks, res.results):
        if r1 > r0:
            y[r0:r1] = out["yT"][:, : r1 - r0].T.astype(np.float32)

    # leftover rows (device capacity is 8 x 1024): host fp32 BLAS
    for e, r0, r1 in host_segs:
        y[r0:r1] = x[r0:r1] @ weight[e].T
    return y
